# revision 1
# baseline (speedup 1.0000x reference)
"""Trainium2 Bass kernel for LoRA linear: y = x @ (W + 2*B@A).T + b.

Full inputs: x (8, 2048, 2048) f32, W (2048, 2048) f32, b (2048,) f32,
B (2048, 16) f32, A (16, 2048) f32.  Output (8, 2048, 2048) f32.

Sharding: data-parallel over the batch dim — core i computes
y[i] = x[i] @ w.T + b with the merged weight w = W + 2*B@A.

Per-core kernel (bf16 TensorEngine compute, f32 accumulate):
  phase 0: cast-DMA A/B to bf16, build 2*B.T via PE transposes,
           broadcast bias, build bf16 identity.
  phase 1: build wT[d, o] = bf16(W.T) + A.T @ (2B).T — bf16 PE transposes
           of cast-DMA'd W tiles (ScalarE evicts PSUM->SBUF), rank-16
           bf16 matmul delta in f32 PSUM added in-place by VectorE.
  phase 2: per 128-row x tile: bf16 PE transposes of the cast-DMA'd
           x tile (ScalarE evicts), then 16x [128,128]x[128,512] bf16
           matmuls per output bank, VectorE adds the bias during
           PSUM->SBUF eviction, DMA out.
"""

import numpy as np

import concourse.bacc as bacc
import concourse.mybir as mybir
import concourse.tile as tile
from concourse import masks
from concourse.bass_utils import run_bass_kernel_spmd
from concourse.tile_rust import add_dep_helper

N_CORES = 8
BATCH, S, D = 8, 2048, 2048
RANK = 16
SCALE = 2.0  # alpha / rank = 32 / 16
P = 128  # partitions
FREE = 512  # f32 elems per PSUM bank
ND = D // P  # 16 contraction tiles
NS = S // P  # 16 row tiles per core
NO = D // FREE  # 4 output banks per row tile
NG = ND // 4  # 4 transpose groups (4x 128-col transposes per PSUM bank)

F32 = mybir.dt.float32
BF16 = mybir.dt.bfloat16


def build_nc():
    nc = bacc.Bacc(
        "TRN2", target_bir_lowering=False, debug=False, num_devices=N_CORES
    )
    x_d = nc.dram_tensor("x", [S, D], F32, kind="ExternalInput").ap()
    W_d = nc.dram_tensor("W", [D, D], F32, kind="ExternalInput").ap()
    b_d = nc.dram_tensor("b", [D], F32, kind="ExternalInput").ap()
    B_d = nc.dram_tensor("B", [D, RANK], F32, kind="ExternalInput").ap()
    A_d = nc.dram_tensor("A", [RANK, D], F32, kind="ExternalInput").ap()
    out_d = nc.dram_tensor("out", [S, D], F32, kind="ExternalOutput").ap()
    # bf16 scratch holding the merged weight w = W + 2*B@A, row-major [o, d]
    Wb_d = nc.dram_tensor("Wb", [D, D], BF16).ap()

    with tile.TileContext(nc) as tc:
        with (
            tc.tile_pool(name="singles", bufs=1) as singles,
            tc.tile_pool(name="wt", bufs=1) as wtp,
        ):
            ident = singles.tile([P, P], BF16)
            masks.make_identity(nc, ident[:])

            A_sb = singles.tile([RANK, D], BF16)
            nc.gpsimd.dma_start(out=A_sb[:], in_=A_d[:])

            # 2 * B.T: cast-load B as [128, (t, r)], PE-transpose, scale
            B2T = singles.tile([RANK, D], BF16)
            Bs = singles.tile([P, ND * RANK], BF16)
            nc.gpsimd.dma_start(
                out=Bs[:], in_=B_d.rearrange("(t p) r -> p t r", p=P)
            )

            # bias replicated across all 128 partitions (needed late —
            # keep it behind A/B in the SWDGE queue)
            bb = singles.tile([P, D], F32)
            nc.gpsimd.dma_start(out=bb[:], in_=b_d[None, :].broadcast_to([P, D]))

            # merged transposed weight, bf16: wT[p, dt, o] = w[o, dt*128+p]
            wT = wtp.tile([P, ND, D], BF16)

            with (
                tc.tile_pool(name="wrow", bufs=3) as wrowp,
                tc.tile_pool(name="w16", bufs=3) as w16p,
                tc.tile_pool(name="xstage", bufs=4) as xstage,
                tc.tile_pool(name="xTp", bufs=5) as xTp,
                tc.tile_pool(name="yout", bufs=2) as youtp,
                tc.tile_pool(name="dpsum", bufs=4, space="PSUM") as dpsum,
                tc.tile_pool(name="tpsum", bufs=2, space="PSUM") as tpsum,
                tc.tile_pool(name="gpsum", bufs=2, space="PSUM") as gpsum,
            ):
                # 2*B.T from the staged B tiles (shares the delta psum slots)
                for g in range(NG):
                    bps = dpsum.tile([RANK, 4 * P], BF16, tag="dp")
                    for j in range(4):
                        t = 4 * g + j
                        nc.tensor.matmul(
                            bps[:, j * P : (j + 1) * P],
                            Bs[:, t * RANK : (t + 1) * RANK],
                            ident[:],
                            is_transpose=True,
                            start=(j == 0),
                            stop=(j == 3),
                        )
                    nc.vector.tensor_scalar_mul(
                        B2T[:, g * 4 * P : (g + 1) * 4 * P], bps[:], SCALE
                    )

                # ---- merged-weight build ----
                # Per 128-row block of W: load f32 rows, compute the rank-16
                # LoRA delta in natural [o, d] orientation on the PE
                # (delta = B2T[:, rows].T @ A), merge + cast on the DVE
                # (w16 = bf16(wrow + delta)), store the bf16 merged rows to
                # DRAM.  Then 16 DMA-xbar transposes produce wT directly.
                def w_chain(ot):
                    # loads on the scalar HWDGE queue, stores (+ transposes,
                    # later) on sync — mixing them in one ring head-of-line
                    # blocks loads behind stores that wait on the DVE merge
                    wrow = wrowp.tile([P, D], F32, tag="wrow")
                    nc.scalar.dma_start(
                        out=wrow[:], in_=W_d[ot * P : (ot + 1) * P, :]
                    )
                    w16 = w16p.tile([P, D], BF16, tag="w16")
                    dps = [
                        dpsum.tile([P, FREE], F32, tag="dp", name=f"dp{ot}_{g}")
                        for g in range(NG)
                    ]
                    for g in range(NG):
                        nc.tensor.matmul(
                            dps[g][:],
                            B2T[:, ot * P : (ot + 1) * P],
                            A_sb[:, g * FREE : (g + 1) * FREE],
                            start=True,
                            stop=True,
                        )
                    for g in range(NG):
                        nc.vector.tensor_add(
                            w16[:, g * FREE : (g + 1) * FREE],
                            dps[g][:],
                            wrow[:, g * FREE : (g + 1) * FREE],
                        )
                    return nc.sync.dma_start(
                        out=Wb_d[ot * P : (ot + 1) * P, :], in_=w16[:]
                    )

                def load_and_transpose_x(st):
                    xs = xstage.tile([P, D], BF16, tag="xs")
                    nc.gpsimd.dma_start(
                        out=xs[:], in_=x_d[st * P : (st + 1) * P, :]
                    )
                    xT = xTp.tile([P, ND, P], BF16, tag="xT")
                    # 8 transposes per bf16 PSUM bank, one ScalarE evict each
                    for g in range(2):
                        tp = tpsum.tile([P, 8 * P], BF16, tag="tp")
                        for j in range(8):
                            dt = 8 * g + j
                            nc.tensor.matmul(
                                tp[:, j * P : (j + 1) * P],
                                xs[:, dt * P : (dt + 1) * P],
                                ident[:],
                                is_transpose=True,
                                start=(j == 0),
                                stop=(j == 7),
                            )
                        nc.scalar.copy(xT[:, 8 * g : 8 * (g + 1), :], tp[:])
                    return xT

                store_insts = [w_chain(ot) for ot in range(ND)]
                # All xbar transposes go on ONE HWDGE queue: concurrent
                # transposes on different queues corrupt each other (shared
                # xbar state); same-queue concurrency is safe.  Full-height
                # transposes all depend on every store, so the scheduler
                # cannot interleave them between the stores (each
                # copy<->transpose xbar mode switch stalls the ring).
                for dt in range(ND):
                    t_inst = nc.sync.dma_start_transpose(
                        out=wT[:, dt, :],
                        in_=Wb_d[:, dt * P : (dt + 1) * P],
                    )
                    for s_inst in store_insts:
                        add_dep_helper(t_inst.ins, s_inst.ins, reason="Wb RAW")

                PRE = 4  # x row-tiles transposed ahead of the GEMM
                xTs = [load_and_transpose_x(st) for st in range(PRE)]

                # ---- main loop: y = x @ wT + b ----
                for st in range(NS):
                    if st + PRE < NS:
                        xTs.append(load_and_transpose_x(st + PRE))
                    xT = xTs[st]
                    ys = youtp.tile([P, D], F32)
                    for oc in range(NO):
                        gp = gpsum.tile([P, FREE], F32)
                        for dt in range(ND):
                            nc.tensor.matmul(
                                gp[:],
                                xT[:, dt, :],
                                wT[:, dt, oc * FREE : (oc + 1) * FREE],
                                start=(dt == 0),
                                stop=(dt == ND - 1),
                            )
                        nc.vector.tensor_add(
                            ys[:, oc * FREE : (oc + 1) * FREE],
                            gp[:],
                            bb[:, oc * FREE : (oc + 1) * FREE],
                        )
                    # y stores on the sync queue: keep the scalar HWDGE queue
                    # clear of copies while transposes may still be in flight
                    nc.sync.dma_start(out=out_d[st * P : (st + 1) * P, :], in_=ys[:])

    nc.compile()
    return nc


_NC_CACHE = None


def _get_nc():
    global _NC_CACHE
    if _NC_CACHE is None:
        _NC_CACHE = build_nc()
    return _NC_CACHE


def make_in_maps(x, W, b, B, A):
    x = np.ascontiguousarray(x, dtype=np.float32)
    W = np.ascontiguousarray(W, dtype=np.float32)
    b = np.ascontiguousarray(b, dtype=np.float32)
    B = np.ascontiguousarray(B, dtype=np.float32)
    A = np.ascontiguousarray(A, dtype=np.float32)
    return [
        {"x": x[i], "W": W, "b": b, "B": B, "A": A} for i in range(N_CORES)
    ]


def run(inputs, **spmd_kwargs):
    """Run the SPMD kernel; returns (output, BassKernelResults)."""
    nc = _get_nc()
    in_maps = make_in_maps(**inputs)
    res = run_bass_kernel_spmd(nc, in_maps, core_ids=list(range(N_CORES)), **spmd_kwargs)
    out = np.stack([res.results[i]["out"] for i in range(N_CORES)]).astype(np.float32)
    return out, res


def kernel(x, W, b, B, A):
    out, _ = run({"x": x, "W": W, "b": b, "B": B, "A": A})
    return out



# revision 3
# speedup vs baseline: 1.1885x; 1.1885x over previous
"""Trainium2 Bass kernel for LoRA linear: y = x @ (W + 2*B@A).T + b.

Full inputs: x (8, 2048, 2048) f32, W (2048, 2048) f32, b (2048,) f32,
B (2048, 16) f32, A (16, 2048) f32.  Output (8, 2048, 2048) f32.

Sharding: data-parallel over the batch dim — core i computes
y[i] = x[i] @ w.T + b with the merged weight w = W + 2*B@A.

Per-core kernel (bf16 TensorEngine compute, f32 accumulate):
  phase 0: cast-DMA A/B/b to bf16, build 2*B.T via PE transposes,
           broadcast the bias across partitions with a rank-1 PE matmul,
           build a bf16 identity.
  phase W: per 128-row block of W (even blocks on the scalar HWDGE
           queue, odd on sync, so the full 16 MiB lands in ~half the
           time): rank-16 delta matmul in f32 PSUM, DVE merge+cast
           w16 = bf16(wrow + delta), then 16 PE transposes of the
           merged rows straight into wT (DVE evicts PSUM->SBUF).
           No DRAM round trip and no all-stores barrier: the old
           xbar-transpose scheme serialized ~60us mid-kernel.
  phase x (interleaved): per 128-row x tile: cast-DMA, 16 PE
           transposes (ScalarE evicts).  x-chains are emitted between
           W-chains so the PE stays busy while W streams in.
  main:    per 128-row tile: 4 PSUM banks x 16 bf16 matmuls,
           VectorE adds the bias during PSUM->SBUF eviction, DMA out
           on the sync queue.
"""

import numpy as np

import concourse.bacc as bacc
import concourse.mybir as mybir
import concourse.tile as tile
from concourse import masks
from concourse.bass_utils import run_bass_kernel_spmd

N_CORES = 8
BATCH, S, D = 8, 2048, 2048
RANK = 16
SCALE = 2.0  # alpha / rank = 32 / 16
P = 128  # partitions
FREE = 512  # f32 elems per PSUM bank
ND = D // P  # 16 contraction tiles
NS = S // P  # 16 row tiles per core
NO = D // FREE  # 4 output banks per row tile
NG = ND // 4  # 4 groups of 4

F32 = mybir.dt.float32
BF16 = mybir.dt.bfloat16


def build_nc():
    nc = bacc.Bacc(
        "TRN2", target_bir_lowering=False, debug=False, num_devices=N_CORES
    )
    x_d = nc.dram_tensor("x", [S, D], F32, kind="ExternalInput").ap()
    W_d = nc.dram_tensor("W", [D, D], F32, kind="ExternalInput").ap()
    b_d = nc.dram_tensor("b", [D], F32, kind="ExternalInput").ap()
    B_d = nc.dram_tensor("B", [D, RANK], F32, kind="ExternalInput").ap()
    A_d = nc.dram_tensor("A", [RANK, D], F32, kind="ExternalInput").ap()
    out_d = nc.dram_tensor("out", [S, D], F32, kind="ExternalOutput").ap()

    with tile.TileContext(nc) as tc:
        with (
            tc.tile_pool(name="singles", bufs=1) as singles,
            tc.tile_pool(name="wt", bufs=1) as wtp,
        ):
            ident = singles.tile([P, P], BF16)
            masks.make_identity(nc, ident[:])
            ones = singles.tile([1, P], BF16)
            nc.vector.memset(ones[:], 1.0)

            A_sb = singles.tile([RANK, D], BF16)
            nc.gpsimd.dma_start(out=A_sb[:], in_=A_d[:])

            # B staged as [128, (t, r)] for the PE transposes below
            B2T = singles.tile([RANK, D], BF16)
            Bs = singles.tile([P, ND * RANK], BF16)
            nc.gpsimd.dma_start(
                out=Bs[:], in_=B_d.rearrange("(t p) r -> p t r", p=P)
            )

            b_sb = singles.tile([1, D], BF16)
            nc.gpsimd.dma_start(out=b_sb[:], in_=b_d[None, :])

            # bias replicated across all 128 partitions (PE rank-1 matmul:
            # a 1 MiB broadcast DMA would block the SWDGE queue ~30us)
            bb = singles.tile([P, D], BF16)

            # merged transposed weight, bf16: wT[p, dt, o] = w[o, dt*128+p]
            wT = wtp.tile([P, ND, D], BF16)

            with (
                tc.tile_pool(name="wrow", bufs=4) as wrowp,
                tc.tile_pool(name="w16", bufs=3) as w16p,
                tc.tile_pool(name="xstage", bufs=5) as xstage,
                tc.tile_pool(name="xTp", bufs=7) as xTp,
                tc.tile_pool(name="yout", bufs=2) as youtp,
                tc.tile_pool(name="dpsum", bufs=3, space="PSUM") as dpsum,
                tc.tile_pool(name="tpsum", bufs=3, space="PSUM") as tpsum,
                tc.tile_pool(name="gpsum", bufs=2, space="PSUM") as gpsum,
            ):
                # 2*B.T from the staged B tiles
                for g in range(NG):
                    bps = tpsum.tile([RANK, 4 * P], BF16, tag="tp")
                    for j in range(4):
                        t = 4 * g + j
                        nc.tensor.matmul(
                            bps[:, j * P : (j + 1) * P],
                            Bs[:, t * RANK : (t + 1) * RANK],
                            ident[:],
                            is_transpose=True,
                            start=(j == 0),
                            stop=(j == 3),
                        )
                    nc.vector.tensor_scalar_mul(
                        B2T[:, g * 4 * P : (g + 1) * 4 * P], bps[:], SCALE
                    )

                # bias broadcast: bb[p, o] = 1[p] * b[o]
                for g in range(NO):
                    bp = dpsum.tile([P, FREE], F32, tag="dp")
                    nc.tensor.matmul(
                        bp[:],
                        ones[:],
                        b_sb[:, g * FREE : (g + 1) * FREE],
                        start=True,
                        stop=True,
                    )
                    nc.scalar.copy(bb[:, g * FREE : (g + 1) * FREE], bp[:])

                # ---- merged-weight build ----
                # Per 128-row block of W: load f32 rows (alternating HWDGE
                # queues), rank-16 LoRA delta on the PE in [o, d]
                # orientation (delta = B2T[:, rows].T @ A), DVE merge+cast
                # w16 = bf16(wrow + delta), then 16 PE transposes of the
                # merged rows; DVE evicts the bf16 PSUM banks into wT.
                def w_chain(ot):
                    wrow = wrowp.tile([P, D], F32, tag="wrow")
                    eng = nc.scalar if ot % 2 == 0 else nc.sync
                    eng.dma_start(
                        out=wrow[:], in_=W_d[ot * P : (ot + 1) * P, :]
                    )
                    w16 = w16p.tile([P, D], BF16, tag="w16")
                    dps = [
                        dpsum.tile([P, FREE], F32, tag="dp", name=f"dp{ot}_{g}")
                        for g in range(NG)
                    ]
                    for g in range(NG):
                        nc.tensor.matmul(
                            dps[g][:],
                            B2T[:, ot * P : (ot + 1) * P],
                            A_sb[:, g * FREE : (g + 1) * FREE],
                            start=True,
                            stop=True,
                        )
                    for g in range(NG):
                        nc.vector.tensor_add(
                            w16[:, g * FREE : (g + 1) * FREE],
                            dps[g][:],
                            wrow[:, g * FREE : (g + 1) * FREE],
                        )
                    # transpose the merged rows: w16[o, dt*128+j] ->
                    # wT[:, dt, ot*128 + o]
                    for g in range(2):
                        tp = tpsum.tile([P, 8 * P], BF16, tag="tp")
                        for j in range(8):
                            dt = 8 * g + j
                            nc.tensor.matmul(
                                tp[:, j * P : (j + 1) * P],
                                w16[:, dt * P : (dt + 1) * P],
                                ident[:],
                                is_transpose=True,
                                start=(j == 0),
                                stop=(j == 7),
                            )
                        nc.vector.tensor_scalar_mul(
                            wT[:, 8 * g : 8 * (g + 1), ot * P : (ot + 1) * P],
                            tp[:],
                            1.0,
                        )

                def load_and_transpose_x(st):
                    xs = xstage.tile([P, D], BF16, tag="xs")
                    nc.gpsimd.dma_start(
                        out=xs[:], in_=x_d[st * P : (st + 1) * P, :]
                    )
                    xT = xTp.tile([P, ND, P], BF16, tag="xT")
                    # 8 transposes per bf16 PSUM bank, one ScalarE evict each
                    for g in range(2):
                        tp = tpsum.tile([P, 8 * P], BF16, tag="tp")
                        for j in range(8):
                            dt = 8 * g + j
                            nc.tensor.matmul(
                                tp[:, j * P : (j + 1) * P],
                                xs[:, dt * P : (dt + 1) * P],
                                ident[:],
                                is_transpose=True,
                                start=(j == 0),
                                stop=(j == 7),
                            )
                        nc.scalar.copy(xT[:, 8 * g : 8 * (g + 1), :], tp[:])
                    return xT

                # Emit W-chains with x-chains interleaved at the points where
                # their data will have arrived (x on the software DGE runs
                # ~4us/tile; W blocks land every ~2.4us across two queues).
                xTs = []
                for ot in range(ND):
                    w_chain(ot)
                    if ot in (3, 6, 9, 12, 15):
                        xTs.append(load_and_transpose_x(len(xTs)))
                xTs.append(load_and_transpose_x(len(xTs)))  # 6 pre-done
                PRE = len(xTs)

                # ---- main loop: y = x @ wT + b ----
                for st in range(NS):
                    if st + PRE < NS:
                        xTs.append(load_and_transpose_x(st + PRE))
                    xT = xTs[st]
                    ys = youtp.tile([P, D], F32)
                    for oc in range(NO):
                        gp = gpsum.tile([P, FREE], F32)
                        for dt in range(ND):
                            nc.tensor.matmul(
                                gp[:],
                                xT[:, dt, :],
                                wT[:, dt, oc * FREE : (oc + 1) * FREE],
                                start=(dt == 0),
                                stop=(dt == ND - 1),
                            )
                        nc.vector.tensor_add(
                            ys[:, oc * FREE : (oc + 1) * FREE],
                            gp[:],
                            bb[:, oc * FREE : (oc + 1) * FREE],
                        )
                    nc.sync.dma_start(out=out_d[st * P : (st + 1) * P, :], in_=ys[:])

    nc.compile()
    return nc


_NC_CACHE = None


def _get_nc():
    global _NC_CACHE
    if _NC_CACHE is None:
        _NC_CACHE = build_nc()
    return _NC_CACHE


def make_in_maps(x, W, b, B, A):
    x = np.ascontiguousarray(x, dtype=np.float32)
    W = np.ascontiguousarray(W, dtype=np.float32)
    b = np.ascontiguousarray(b, dtype=np.float32)
    B = np.ascontiguousarray(B, dtype=np.float32)
    A = np.ascontiguousarray(A, dtype=np.float32)
    return [
        {"x": x[i], "W": W, "b": b, "B": B, "A": A} for i in range(N_CORES)
    ]


def run(inputs, **spmd_kwargs):
    """Run the SPMD kernel; returns (output, BassKernelResults)."""
    nc = _get_nc()
    in_maps = make_in_maps(**inputs)
    res = run_bass_kernel_spmd(nc, in_maps, core_ids=list(range(N_CORES)), **spmd_kwargs)
    out = np.stack([res.results[i]["out"] for i in range(N_CORES)]).astype(np.float32)
    return out, res


def kernel(x, W, b, B, A):
    out, _ = run({"x": x, "W": W, "b": b, "B": B, "A": A})
    return out
